# revision 48
# baseline (speedup 1.0000x reference)
"""GQA attention Trainium2 kernel (8 NeuronCores, SPMD, no collectives).

Sharding: 2-way data parallel (batch) x 4-way tensor parallel (heads).
Core c handles batch b=c//4 and head-group g=c%4 (8 q heads, 2 kv heads).
Each core produces a partial o_proj output (transposed, [HID, S] bf16);
the host sums the 4 partials per batch and transposes back.

V3: single fused pipeline, emission order proj(0); for sb: attn(sb),
proj(sb+1), o_proj(sb) so the PE always has dense runnable work.
Attention is software-pipelined (PV lags QK/exp by 2 key-chunks) so the
in-order PE queue never head-of-line blocks on the softmax Exp.  All ACT
work uses one table set (exp_and_others): softmax Exp + tanh-sigmoid.
RMS-norm rstd = magic-rsqrt + 2 Newton steps on DVE, batched per block;
rstd broadcast via tiny PE matmuls.  RoPE rotate via DVE stream_shuffle.
Softmax denominators via V-ones column + DVE reciprocal_approx_fast.
PSUM: mm(2) + sc(2x1) + av(2x2) = 8 banks.
"""

import os
import sys
import numpy as np

for _p in ("/opt/trn_rl_repo", "/root/.axon_site/_ro/trn_rl_repo"):
    if os.path.isdir(_p) and _p not in sys.path:
        sys.path.insert(0, _p)

import ml_dtypes

B, S, HID = 2, 2048, 2048
NH, NKV, HD = 32, 8, 64
ROPE = 32
EPS = 1e-6
SCALE = HD ** -0.5
NCORES = 8
QH = NH // 4      # 8 q heads per core
KVH = NKV // 4    # 2 kv heads per core
QD = QH * HD      # 512 per-core q dim
KD = KVH * HD     # 128 per-core kv dim
KC = HID // 128   # 16 contraction chunks
SB = S // 512     # 4 sequence blocks of 512
BF16 = ml_dtypes.bfloat16
MAGIC = 0x5F3759DF

_CACHE = {}

SEL2 = np.zeros((2, 128), BF16)
SEL2[0, 0:64] = 1
SEL2[1, 64:128] = 1


def _build_bass(debug_dump=False):
    import concourse.bass as bass
    from concourse import bacc, mybir, tile
    from concourse.alu_op_type import AluOpType

    f32 = mybir.dt.float32
    bf16 = mybir.dt.bfloat16
    i32 = mybir.dt.int32

    nc = bacc.Bacc("TRN2", target_bir_lowering=False, debug=False,
                   enable_asserts=False, num_devices=NCORES)

    hT = nc.dram_tensor("hT", [HID, S], bf16, kind="ExternalInput").ap()
    wqT = nc.dram_tensor("wqT", [HID, QD], bf16, kind="ExternalInput").ap()
    wkT = nc.dram_tensor("wkT", [HID, KD], bf16, kind="ExternalInput").ap()
    wvT = nc.dram_tensor("wvT", [HID, KD], bf16, kind="ExternalInput").ap()
    wgT = nc.dram_tensor("wgT", [HID, QD], bf16, kind="ExternalInput").ap()
    woT = nc.dram_tensor("woT", [QD, HID], bf16, kind="ExternalInput").ap()
    csAq = nc.dram_tensor("csAq", [128, S], bf16, kind="ExternalInput").ap()
    csBq = nc.dram_tensor("csBq", [128, S], bf16, kind="ExternalInput").ap()
    csAk = nc.dram_tensor("csAk", [128, S], bf16, kind="ExternalInput").ap()
    csBk = nc.dram_tensor("csBk", [128, S], bf16, kind="ExternalInput").ap()
    sel2d = nc.dram_tensor("sel2d", [2, 128], bf16, kind="ExternalInput").ap()
    outT = nc.dram_tensor("outT", [HID, S], bf16, kind="ExternalOutput").ap()
    if debug_dump:
        dbg = {n: nc.dram_tensor(f"dbg_{n}", shp, mybir.dt.bfloat16,
                                 kind="ExternalOutput").ap()
               for n, shp in [("q", [128, 4, S]), ("k", [128, S]),
                              ("g", [128, 4, S]), ("v", [128, KC, KVH, HD + 1]),
                              ("og", [128, 4, S]), ("rstd", [2, 5, 512])]}

    Exp = mybir.ActivationFunctionType.Exp
    Tanh = mybir.ActivationFunctionType.Tanh
    PSUM = bass.MemorySpace.PSUM
    # stream_shuffle mask: swap 16-row halves within each 32-row group
    ROT_MASK = list(range(16, 32)) + list(range(16))

    with tile.TileContext(nc) as tc:
        with tc.tile_pool(name="persist", bufs=1) as pp, \
             tc.tile_pool(name="hblk", bufs=2) as hp, \
             tc.tile_pool(name="rope", bufs=2) as rp, \
             tc.tile_pool(name="qa", bufs=5) as qap, \
             tc.tile_pool(name="sqp", bufs=1) as sqp, \
             tc.tile_pool(name="probs", bufs=6) as prp, \
             tc.tile_pool(name="attsm", bufs=1) as asm, \
             tc.tile_pool(name="attsm2", bufs=2) as asm2, \
             tc.tile_pool(name="ostg", bufs=2) as ostg, \
             tc.tile_pool(name="mm", bufs=2, space=PSUM) as mmp, \
             tc.tile_pool(name="sc", bufs=2, space=PSUM) as scp, \
             tc.tile_pool(name="av", bufs=2, space=PSUM) as avp:

            # ---------------- persistent state ----------------
            qT_sb = pp.tile([128, 4, S], bf16)       # roped+normed q
            kT_sb = pp.tile([128, S], bf16)          # roped+normed k
            g_sb = pp.tile([128, 4, S], bf16)        # sigmoid(gate)
            v_sb = pp.tile([128, KC, KVH, HD + 1], bf16)  # natural V + ones
            og_sb = pp.tile([128, 4, S], bf16)       # gated attn out
            wo_sb = pp.tile([128, 4, KC, 128], bf16)
            wq_sb = pp.tile([128, KC, QD], bf16)
            wk_sb = pp.tile([128, KC, KD], bf16)
            wv_sb = pp.tile([128, KC, KD], bf16)
            wg_sb = pp.tile([128, KC, QD], bf16)
            csA_q = pp.tile([128, S], bf16)
            csB_q = pp.tile([128, S], bf16)
            csA_k = pp.tile([128, S], bf16)
            csB_k = pp.tile([128, S], bf16)

            # first block's activations + q-path weights first so the PE
            # can start ASAP; the rest follows.
            ha0 = hp.tile([128, 8, 512], bf16, tag="hblk")
            hb0 = hp.tile([128, 8, 512], bf16, tag="hblk")
            nc.sync.dma_start(
                out=ha0, in_=hT[0:1024, 0:512].rearrange("(c p) s -> p c s",
                                                         p=128))
            nc.sync.dma_start(
                out=hb0, in_=hT[1024:2048, 0:512].rearrange("(c p) s -> p c s",
                                                            p=128))
            wq_view = wqT.rearrange("(c p) m -> p c m", p=128)
            for m in range(4):
                nc.sync.dma_start(out=wq_sb[:, :, m * 128:(m + 1) * 128],
                                  in_=wq_view[:, :, m * 128:(m + 1) * 128])
            nc.sync.dma_start(out=wk_sb,
                              in_=wkT.rearrange("(c p) m -> p c m", p=128))
            nc.sync.dma_start(out=csA_q, in_=csAq)
            nc.sync.dma_start(out=csB_q, in_=csBq)
            nc.sync.dma_start(out=csA_k, in_=csAk)
            nc.sync.dma_start(out=csB_k, in_=csBk)
            nc.sync.dma_start(out=wv_sb,
                              in_=wvT.rearrange("(c p) m -> p c m", p=128))
            nc.sync.dma_start(out=wg_sb,
                              in_=wgT.rearrange("(c p) m -> p c m", p=128))
            nc.sync.dma_start(out=wo_sb,
                              in_=woT.rearrange("(c p) (mb mm) -> p c mb mm",
                                                p=128, mm=128))

            ident = pp.tile([128, 128], bf16)
            from concourse.masks import make_identity
            make_identity(nc, ident)
            ones2 = pp.tile([128, 2], bf16)
            nc.vector.memset(ones2, 0.0)
            nc.vector.memset(ones2[0:64, 0:1], 1.0)
            nc.vector.memset(ones2[64:128, 1:2], 1.0)
            sel2 = pp.tile([2, 128], bf16)           # broadcast selector
            nc.sync.dma_start(out=sel2, in_=sel2d)
            nc.vector.memset(v_sb[:, :, :, HD:HD + 1], 1.0)

            # per-sb rstd staging (persistent; reused each sb serially)
            sqs = pp.tile([2, 2, 512], f32)          # sumsq staging (2 slots)
            sqg = pp.tile([16, 512], f32)            # partition-major sumsq
            rstd_bf = pp.tile([16, 512], bf16)       # final 8/sqrt(ms)
            rstd_sep = pp.tile([2, 5, 512], bf16)    # redistributed

            def proj_chunk(ha, hb, w_sb, msl):
                ps = mmp.tile([128, 512], f32, tag="mm")
                for kc in range(KC):
                    h = ha if kc < 8 else hb
                    nc.tensor.matmul(ps, w_sb[:, kc, msl],
                                     h[:, kc % 8, :],
                                     start=(kc == 0), stop=(kc == KC - 1))
                return ps

            def rope_block(ps, csA, csB, c):
                """bf16 rope on sbuf; emits sumsq gather into sqs[:,c,:]."""
                raw = rp.tile([128, 512], bf16, tag="raw")
                nc.vector.tensor_copy(out=raw, in_=ps)
                sq = rp.tile([128, 512], bf16, tag="sq")
                nc.vector.tensor_tensor(out=sq, in0=raw, in1=raw,
                                        op=AluOpType.mult)
                sq_ps = mmp.tile([2, 512], f32, tag="mm")
                nc.tensor.matmul(sq_ps, ones2, sq, start=True, stop=True)
                nc.vector.tensor_scalar(out=sqs[:, c % 2, :], in0=sq_ps,
                                        scalar1=float(HD) * EPS, scalar2=None,
                                        op0=AluOpType.add)
                nc.gpsimd.dma_start(out=sqg[2 * c:2 * c + 2, :],
                                    in_=sqs[:, c % 2, :])
                rot = rp.tile([128, 512], bf16, tag="rot")
                nc.vector.stream_shuffle(rot, raw, ROT_MASK)
                t1 = rp.tile([128, 512], bf16, tag="t1")
                nc.vector.tensor_tensor(out=t1, in0=raw, in1=csA,
                                        op=AluOpType.mult)
                t2 = rp.tile([128, 512], bf16, tag="t2")
                nc.vector.tensor_tensor(out=t2, in0=rot, in1=csB,
                                        op=AluOpType.mult)
                qa2 = qap.tile([128, 512], bf16, tag="qa2")
                nc.vector.tensor_add(qa2, t1, t2)
                return qa2

            def rstd_chain():
                """sqs[2,5,512] -> rstd_sep[2,5,512] (=8/sqrt(ms), bf16)"""
                ms = sqg[0:10, :]
                ms_i = ms.bitcast(i32)
                sh = sqp.tile([16, 512], i32, tag="sh")
                nc.vector.tensor_scalar(out=sh[0:10], in0=ms_i, scalar1=1,
                                        scalar2=None,
                                        op0=AluOpType.logical_shift_right)
                y = sqp.tile([16, 512], f32, tag="y")
                y_i = y.bitcast(i32)
                # y0 bits = MAGIC - (i >> 1)
                nc.vector.tensor_scalar(out=y_i[0:10], in0=sh[0:10],
                                        scalar1=-1, scalar2=MAGIC,
                                        op0=AluOpType.mult,
                                        op1=AluOpType.add)
                for _ in range(2):   # Newton iterations for rsqrt
                    a = sqp.tile([16, 512], f32, tag="nra")
                    nc.vector.tensor_tensor(out=a[0:10], in0=y[0:10],
                                            in1=y[0:10], op=AluOpType.mult)
                    nc.vector.tensor_tensor(out=a[0:10], in0=a[0:10],
                                            in1=ms, op=AluOpType.mult)
                    nc.vector.tensor_scalar(out=a[0:10], in0=a[0:10],
                                            scalar1=-0.5, scalar2=1.5,
                                            op0=AluOpType.mult,
                                            op1=AluOpType.add)
                    nc.vector.tensor_tensor(out=y[0:10], in0=y[0:10],
                                            in1=a[0:10], op=AluOpType.mult)
                nc.vector.tensor_scalar(out=rstd_bf[0:10], in0=y[0:10],
                                        scalar1=float(HD) ** 0.5, scalar2=None,
                                        op0=AluOpType.mult)
                for c in range(5):
                    nc.gpsimd.dma_start(out=rstd_sep[:, c, :],
                                        in_=rstd_bf[2 * c:2 * c + 2, :])

            def emit_proj(sb, ha, hb):
                s0 = sb * 512
                ssl = slice(s0, s0 + 512)
                qa_list = []
                for m in range(4):
                    ps = proj_chunk(ha, hb, wq_sb, slice(m * 128, (m + 1) * 128))
                    qa_list.append(rope_block(ps, csA_q[:, ssl],
                                              csB_q[:, ssl], m))
                psk = proj_chunk(ha, hb, wk_sb, slice(0, 128))
                ka2 = rope_block(psk, csA_k[:, ssl], csB_k[:, ssl], 4)

                rstd_chain()     # DVE, overlaps the v/g proj below

                psv = proj_chunk(ha, hb, wv_sb, slice(0, 128))
                vt = rp.tile([128, 512], bf16, tag="t2")
                nc.vector.tensor_copy(out=vt, in_=psv)
                for ss in range(4):
                    tp = mmp.tile([128, 128], bf16, tag="mm")
                    nc.tensor.transpose(tp, vt[:, ss * 128:(ss + 1) * 128],
                                        ident)
                    chunk = sb * 4 + ss
                    nc.vector.tensor_copy(out=v_sb[:, chunk, 0, 0:HD],
                                          in_=tp[:, 0:64])
                    nc.vector.tensor_copy(out=v_sb[:, chunk, 1, 0:HD],
                                          in_=tp[:, 64:128])

                for m in range(4):
                    ps = proj_chunk(ha, hb, wg_sb, slice(m * 128, (m + 1) * 128))
                    th = rp.tile([128, 512], bf16, tag="t1")
                    nc.scalar.activation(out=th, in_=ps, func=Tanh, scale=0.5)
                    nc.vector.tensor_scalar(out=g_sb[:, m, ssl], in0=th,
                                            scalar1=0.5, scalar2=0.5,
                                            op0=AluOpType.mult,
                                            op1=AluOpType.add)

                # apply rstd: qT/kT = qa2 * bcast(rstd)
                for c in range(4):
                    rb_ps = mmp.tile([128, 512], f32, tag="mm")
                    nc.tensor.matmul(rb_ps, sel2, rstd_sep[:, c, :],
                                     start=True, stop=True)
                    r = (c // 2) * 64
                    cb = 2 * (c % 2)
                    nc.vector.tensor_tensor(
                        out=qT_sb[r:r + 64, cb, ssl],
                        in0=qa_list[c][0:64, :], in1=rb_ps[0:64, :],
                        op=AluOpType.mult)
                    nc.vector.tensor_tensor(
                        out=qT_sb[r:r + 64, cb + 1, ssl],
                        in0=qa_list[c][64:128, :], in1=rb_ps[64:128, :],
                        op=AluOpType.mult)
                rb_ps = mmp.tile([128, 512], f32, tag="mm")
                nc.tensor.matmul(rb_ps, sel2, rstd_sep[:, 4, :],
                                 start=True, stop=True)
                nc.vector.tensor_tensor(out=kT_sb[:, ssl], in0=ka2,
                                        in1=rb_ps, op=AluOpType.mult)

            def emit_attn(sb):
                """PV lags QK/exp by 2 chunks so the PE stream never waits
                on the ACT exp."""
                s0 = sb * 512
                ssl = slice(s0, s0 + 512)
                nkc = 4 * (sb + 1)
                for duo in range(4):
                    hA = 2 * duo
                    kv = hA // 4
                    r0 = kv * 64
                    av = avp.tile([65, 2, 512], f32, tag="av")
                    live = {}
                    for t in range(nkc + 2):
                        if t < nkc:
                            pr2 = []
                            for hh in range(2):
                                sc = scp.tile([128, 512], f32, tag="sc")
                                qc = (hA + hh) % 4
                                nc.tensor.matmul(
                                    sc,
                                    kT_sb[r0:r0 + 64,
                                          t * 128:(t + 1) * 128],
                                    qT_sb[r0:r0 + 64, qc, ssl],
                                    start=True, stop=True)
                                probs = prp.tile([128, 512], bf16,
                                                 tag="probs")
                                nc.scalar.activation(out=probs, in_=sc,
                                                     func=Exp, scale=SCALE)
                                if t >= 4 * sb:
                                    nc.gpsimd.affine_select(
                                        out=probs, in_=probs,
                                        compare_op=mybir.AluOpType.is_ge,
                                        fill=0.0,
                                        base=s0 - t * 128,
                                        channel_multiplier=-1,
                                        pattern=[[1, 512]])
                                pr2.append(probs)
                            live[t] = pr2
                        if t >= 2:
                            kc = t - 2
                            pr2 = live.pop(kc)
                            for hh in range(2):
                                nc.tensor.matmul(
                                    av[:, hh, :],
                                    v_sb[:, kc, kv, :],
                                    pr2[hh],
                                    start=(kc == 0), stop=(kc == nkc - 1))
                    # drain: og = (av / denom) * gate
                    recip = asm.tile([1, 2, 512], f32, tag="recip")
                    rbv = asm.tile([64, 2, 512], f32, tag="rbv")
                    nc.vector.tensor_copy(
                        out=rbv[0:1, :, :].rearrange("p a b -> p (a b)"),
                        in_=av[64:65, :, :].rearrange("p a b -> p (a b)"))
                    nc.vector.reciprocal_approx_fast(
                        out=recip.rearrange("p a b -> p (a b)"),
                        in_=rbv[0:1, :, :].rearrange("p a b -> p (a b)"))
                    nc.gpsimd.partition_broadcast(
                        rbv.rearrange("p a b -> p (a b)"),
                        recip.rearrange("p a b -> p (a b)"))
                    t128 = asm2.tile([128, 512], bf16, tag="t")
                    for hh in range(2):
                        nc.vector.tensor_tensor(
                            out=t128[hh * 64:hh * 64 + 64, :],
                            in0=av[0:64, hh, :], in1=rbv[:, hh, :],
                            op=AluOpType.mult)
                    nc.vector.tensor_tensor(
                        out=og_sb[:, duo, ssl],
                        in0=t128, in1=g_sb[:, duo, ssl],
                        op=AluOpType.mult)

            def emit_oproj(sb):
                ssl = slice(sb * 512, (sb + 1) * 512)
                for m in range(KC):
                    po = mmp.tile([128, 512], f32, tag="mm")
                    for oc in range(4):
                        nc.tensor.matmul(po, wo_sb[:, oc, m, :],
                                         og_sb[:, oc, ssl],
                                         start=(oc == 0), stop=(oc == 3))
                    stg = ostg.tile([128, 512], bf16, tag="stg")
                    if m % 2 == 0:
                        nc.scalar.copy(out=stg, in_=po)
                    else:
                        nc.vector.tensor_copy(out=stg, in_=po)
                    nc.sync.dma_start(
                        out=outT[m * 128:(m + 1) * 128, ssl],
                        in_=stg)

            # ================= fused pipeline =================
            emit_proj(0, ha0, hb0)
            for sb in range(SB):
                if sb < SB - 1:
                    s1 = (sb + 1) * 512
                    ha = hp.tile([128, 8, 512], bf16, tag="hblk")
                    hb = hp.tile([128, 8, 512], bf16, tag="hblk")
                    nc.sync.dma_start(
                        out=ha,
                        in_=hT[0:1024, s1:s1 + 512].rearrange(
                            "(c p) s -> p c s", p=128))
                    nc.sync.dma_start(
                        out=hb,
                        in_=hT[1024:2048, s1:s1 + 512].rearrange(
                            "(c p) s -> p c s", p=128))
                emit_attn(sb)
                if sb < SB - 1:
                    emit_proj(sb + 1, ha, hb)
                emit_oproj(sb)

            if debug_dump:
                nc.sync.dma_start(out=dbg["q"], in_=qT_sb)
                nc.sync.dma_start(out=dbg["k"], in_=kT_sb)
                nc.sync.dma_start(out=dbg["g"], in_=g_sb)
                nc.sync.dma_start(out=dbg["v"], in_=v_sb)
                nc.sync.dma_start(out=dbg["og"], in_=og_sb)
                nc.sync.dma_start(out=dbg["rstd"], in_=rstd_sep)

    nc.compile()
    return nc


def _host_prep(hidden_states, cos, sin, Wq, Wk, Wv, Wg, Wo, q_norm_w, k_norm_w):
    """Build per-core input maps."""

    def cs_tables(cos_b, sin_b, w):
        # csA/csB [128, S]: row p -> head-local dim d = p % 64
        A = np.empty((128, S), np.float32)
        Bt = np.empty((128, S), np.float32)
        cosT = cos_b.T  # [32, S]
        sinT = sin_b.T
        for blk in (0, 64):
            A[blk + 0:blk + 32] = cosT * w[0:32, None]
            A[blk + 32:blk + 64] = w[32:64, None]
            Bt[blk + 0:blk + 16] = -sinT[0:16] * w[16:32, None]
            Bt[blk + 16:blk + 32] = sinT[16:32] * w[0:16, None]
            Bt[blk + 32:blk + 64] = 0.0
        return A.astype(BF16), Bt.astype(BF16)

    in_maps = []
    for c in range(NCORES):
        b, g = c // 4, c % 4
        qs = slice(g * QD, (g + 1) * QD)
        ks = slice(g * KD, (g + 1) * KD)
        csA_q, csB_q = cs_tables(cos[b], sin[b], np.asarray(q_norm_w))
        csA_k, csB_k = cs_tables(cos[b], sin[b], np.asarray(k_norm_w))
        in_maps.append({
            "hT": np.ascontiguousarray(hidden_states[b].T).astype(BF16),
            "wqT": np.ascontiguousarray(Wq[qs].T).astype(BF16),
            "wkT": np.ascontiguousarray(Wk[ks].T).astype(BF16),
            "wvT": np.ascontiguousarray(Wv[ks].T).astype(BF16),
            "wgT": np.ascontiguousarray(Wg[qs].T).astype(BF16),
            "woT": np.ascontiguousarray(Wo[:, qs].T).astype(BF16),
            "csAq": csA_q, "csBq": csB_q, "csAk": csA_k, "csBk": csB_k,
            "sel2d": SEL2,
        })
    return in_maps


def kernel(hidden_states, cos, sin, Wq, Wk, Wv, Wg, Wo, q_norm_w, k_norm_w):
    from concourse import bass_utils

    if "nc" not in _CACHE:
        _CACHE["nc"] = _build_bass()
    nc = _CACHE["nc"]

    in_maps = _host_prep(hidden_states, cos, sin, Wq, Wk, Wv, Wg, Wo,
                         q_norm_w, k_norm_w)

    trace = bool(int(os.environ.get("KERNEL_TRACE", "0")))
    kwargs = {}
    if trace:
        # the agent image's antenv lacks axon_hooks; recreate it from the
        # boot helper so run_bass_kernel_spmd(trace=True) can NTFF-profile
        try:
            import antenv.axon_hooks  # noqa: F401
        except ImportError:
            import types
            sys.path.insert(0, "/root/.axon_site")
            from trn_agent_boot.trn_boot import _ntff_profile_via_ctypes
            hook = _ntff_profile_via_ctypes("/opt/axon/libaxon_pjrt.so")
            mod = types.ModuleType("antenv.axon_hooks")
            mod.get_axon_ntff_profile_hook = lambda: hook
            sys.modules["antenv.axon_hooks"] = mod
        tmpdir = os.environ.get("KERNEL_TRACE_DIR") or None
        kwargs = dict(trace=True, tmpdir=tmpdir)
    res = bass_utils.run_bass_kernel_spmd(nc, in_maps,
                                          core_ids=list(range(NCORES)),
                                          **kwargs)
    if trace and res.exec_time_ns is not None:
        print(f"HW exec time: {res.exec_time_ns} ns")
        _CACHE["exec_time_ns"] = res.exec_time_ns

    out = np.zeros((B, S, HID), np.float32)
    for c in range(NCORES):
        b = c // 4
        out[b] += res.results[c]["outT"].astype(np.float32).T
    return out


if __name__ == "__main__":
    rng = np.random.default_rng(0)
    hs = rng.standard_normal((B, S, HID), dtype=np.float32)
    cos = rng.random((B, S, ROPE), dtype=np.float32)
    sin = rng.random((B, S, ROPE), dtype=np.float32)
    out = kernel(hidden_states=hs, cos=cos, sin=sin,
                 Wq=rng.standard_normal((NH * HD, HID), dtype=np.float32) * 0.02,
                 Wk=rng.standard_normal((NKV * HD, HID), dtype=np.float32) * 0.02,
                 Wv=rng.standard_normal((NKV * HD, HID), dtype=np.float32) * 0.02,
                 Wg=rng.standard_normal((NH * HD, HID), dtype=np.float32) * 0.02,
                 Wo=rng.standard_normal((HID, NH * HD), dtype=np.float32) * 0.02,
                 q_norm_w=np.ones(HD, np.float32),
                 k_norm_w=np.ones(HD, np.float32))
    print(out.shape, out.dtype)
